# revision 33
# baseline (speedup 1.0000x reference)
"""MultiHeadedAttention Trainium2 kernel (8-core SPMD, batch x head-half).

Sharding: 8 cores = (batch b in 0..3) x (head-half h in 0..1). Each core
computes partial_h[b] = x_attn[:, h*512:(h+1)*512] @ Wo.T[h*512:] (+ its
share of the v-bias/output-bias row R, computed on host); host sums the
two partials per batch. No collectives.

Host prep pre-transposes and casts activations to bf16 (xT layouts with
the contraction dim on partitions) so the kernel DMAs straight into SBUF
with no staging round-trips.

Per-core dataflow:
  - projections (bf16 matmuls, fp32 psum): qT/kT [d_head, S] per head
    pair, v natural [Sk, d] with a trailing ones column per head (Z trick)
  - scores^T [Sk-tile, Sq-slice] = kT.T @ qT per head (K=64, two heads
    row-tiled concurrently on HW); exp on ScalarE (scale 1/8 folded into
    Wq/bq, no max-subtraction needed: |s/8| <~ 2 by construction); mask
    applied as bf16 multiply with a persistent per-sq-half mask buffer
    (each mask byte is DMAed once, not once per head-pair block)
  - PV: psum rows 0..63 = sum_j v^T p, row 64 = Z (ones col); finalize:
    Z -> sbuf, 1/Z in place (DVE), GpSimd partition-broadcast, multiply,
    SBUF->SBUF DMA hop into head-pair layout xattnT [dout, Sq]
  - partial out = xattnT.T @ WoT_half + R, fp32

Scheduling: ScalarE's exp stream (33.5M elems per core, single-engine,
~265us busy) is the kernel's floor; everything else is arranged to keep
it fed. Only kproj/qproj for (hp0, first Sq half) run before the
attention loop, so exp starts ~14us in. All other projections, the v
projection, R and half the output projection are interleaved into the
attention j-loops as ~1.7us units at explicit (block, j) slots chosen
against their DMA arrival and consumer deadlines. x activations stream
through rotating 1MB quarter buffers (xk x3, xq x2, xv x2) so SBUF fits
the persistent mask half. Each block's softmax finalize is deferred into
the successor block's pipeline (split per head across j=1/j=2) so block
transitions never serialize. Pools and persistent tiles are shared
across reps and the tail output projection runs on the "pv" psum slots,
so back-to-back kernel bodies pipeline through data deps alone (no
inter-rep barrier; a following rep's projections overlap this rep's
tail).
"""
import numpy as np
import ml_dtypes

import concourse.bass as bass
import concourse.mybir as mybir
import concourse.tile as tile
from concourse import bacc
from concourse.bass_utils import run_bass_kernel_spmd

F32 = mybir.dt.float32
BF16 = mybir.dt.bfloat16
I32 = mybir.dt.int32
AF = mybir.ActivationFunctionType
ALU = mybir.AluOpType

N_CORES = 8
DK = 64


def slices(total, chunk):
    return [(s, min(chunk, total - s)) for s in range(0, total, chunk)]


class Cfg:
    def __init__(self, SQ=2048, SK=2048, DM=1024, H=8, max_stage=5):
        assert DM % 128 == 0 and SK % 128 == 0 and SQ % 128 == 0 and H % 2 == 0
        self.SQ, self.SK, self.DM, self.H = SQ, SK, DM, H
        self.DO = H * DK             # per-core projection output dim (512)
        self.KT = DM // 128          # dm contraction chunks (input dim)
        self.KO = self.DO // 128     # output-proj contraction chunks
        self.HP = H // 2             # head pairs
        self.NJ = SK // 128          # Sk tiles
        self.SQS = min(1024, SQ)     # attention Sq slice width (2 psum banks)
        self.max_stage = max_stage   # debug: truncate kernel after stage N


def emit_kernel(tc, cfg, io, state=None):
    nc = tc.nc
    C = cfg
    xqT, xkT, xvT, maskT = io["xqT"], io["xkT"], io["xvT"], io["maskT"]
    w_dram = {"q": io["wqt"], "k": io["wkt"], "v": io["wvt"], "o": io["wot"]}
    bql, bkl, rrow = io["bql"], io["bkl"], io["rrow"]
    out = io["out"]

    # pools + persistent tiles are created once and shared across reps so
    # repeated bodies pipeline through data deps (no inter-rep barrier)
    if state is None:
        state = {}
    if not state:
        pools = state["pools"] = {}

        def open_pool(name, bufs=1, space="SBUF"):
            pools[name] = tc.alloc_tile_pool(name=name, bufs=bufs, space=space)
            return pools[name]

        persist = open_pool("persist", 1)
        # 8 banks: "s" 2 slots x 2 banks + "pv" 2 slots x 2 banks
        open_pool("ps_s", 2, space="PSUM")
        open_pool("ps_pv", 2, space="PSUM")
        t = state["tiles"] = {}
        t["qT_sb"] = persist.tile([128, C.HP * C.SQS], BF16, name="qT_sb")
        t["kT_sb"] = persist.tile([128, C.HP * C.SK], BF16, name="kT_sb")
        t["v_sb"] = persist.tile([128, C.NJ * C.H * 65], BF16, name="v_sb")
        t["mask_sb"] = persist.tile([128, C.NJ * C.SQS], BF16, name="mask_sb")
        t["xattnT_sb"] = persist.tile([128, C.HP * C.SQ], BF16,
                                      name="xattnT_sb")
        t["R_sb"] = persist.tile([128, C.DM], BF16, name="R_sb")
        t["bql_sb"] = persist.tile([128, C.HP], F32, name="bql_sb")
        t["bkl_sb"] = persist.tile([128, C.HP], F32, name="bkl_sb")
        t["wo_sb"] = persist.tile([128, C.KO * C.DM], BF16, name="wo_sb")
        t["wq_sb"] = persist.tile([128, C.KT * C.DO], BF16, name="wq_sb")
        t["wk_sb"] = persist.tile([128, C.KT * C.DO], BF16, name="wk_sb")
        t["wv_sb"] = persist.tile([128, C.KT * C.DO], BF16, name="wv_sb")
        open_pool("stream", 1)
        open_pool("attn", 1)
        vv = t["v_sb"].rearrange("p (j h c) -> p j h c", j=C.NJ, c=65)
        nc.vector.memset(vv[:, :, :, 64:65], 1.0)  # Z ones col

    pools = state["pools"]
    t = state["tiles"]
    ps_s, ps_pv = pools["ps_s"], pools["ps_pv"]
    stream, attn = pools["stream"], pools["attn"]
    qT_sb, kT_sb, v_sb = t["qT_sb"], t["kT_sb"], t["v_sb"]
    mask_sb, xattnT_sb, R_sb = t["mask_sb"], t["xattnT_sb"], t["R_sb"]
    bql_sb, bkl_sb = t["bql_sb"], t["bkl_sb"]
    wo_sb, wq_sb, wk_sb, wv_sb = t["wo_sb"], t["wq_sb"], t["wk_sb"], t["wv_sb"]

    v_view = v_sb.rearrange("p (j h c) -> p j h c", j=C.NJ, c=65)
    PS_F = max(C.SQS, 512)  # tag-"s" psum slot free-size (2 banks at 1024)

    # ---------------- DMA helpers ----------------
    def load_stripe(dma, dst, src_t, S, c0, c1):
        # columns [c0:c1) of every kt block: [KT*128, S] dram -> [128, KT*S]
        dma(dst.rearrange("p (kt s) -> p kt s", s=S)[:, :, c0:c1],
            src_t.rearrange("(kt p) s -> p kt s", p=128)[:, :, c0:c1])

    xbuf = {"k": {}, "q": {}, "v": {}}
    XB = {"k": ("xkq", 3, xkT, nc.scalar.dma_start),
          "q": ("xqq", 2, xqT, nc.sync.dma_start),
          "v": ("xvq", 2, xvT, nc.gpsimd.dma_start)}

    def x_quarter(which, quar, dma=None):
        """Stream x columns [quar*512, (quar+1)*512) of every kt chunk into
        a rotating buffer: [128, KT*512] with kt-major free layout."""
        tag, bufs, src, qdma = XB[which]
        xt = stream.tile([128, C.KT * 512], BF16, name=tag, tag=tag, bufs=bufs,
                         padded_shape=[128, C.KT * 512])
        (dma or qdma)(
            xt.rearrange("p (kt s) -> p kt s", s=512),
            src.rearrange("(kt p) s -> p kt s", p=128)
            [:, :, quar * 512:(quar + 1) * 512],
        )
        xbuf[which][quar] = xt

    JG = 4  # mask DMA group: JG j-tiles per load

    def mask_load(half, g):
        """Load mask j-group g of sq-half `half` into the persistent mask
        buffer (shared by that half's 4 head-pair blocks -- each group is
        DMAed once per half, not once per block). Host layout is
        [(half, j, 128), SQS] so a group is one contiguous block."""
        base = half * C.SK + g * JG * 128
        nc.sync.dma_start(
            mask_sb.rearrange("p (j q) -> p j q", j=C.NJ)
            [:, g * JG:(g + 1) * JG, :],
            maskT[base:base + JG * 128, :].rearrange("(a p) q -> p a q", p=128),
        )

    # startup-critical order (first-use): hp0 weight stripes + first x
    # quarters feed kproj/qproj(hp0, sq0); mask g0 + v group 0 right after
    nc.sync.dma_start(bql_sb[:], bql[:])
    nc.sync.dma_start(bkl_sb[:], bkl[:])
    load_stripe(nc.scalar.dma_start, wk_sb, w_dram["k"], C.DO, 0, 128)
    x_quarter("k", 0)
    load_stripe(nc.sync.dma_start, wq_sb, w_dram["q"], C.DO, 0, 128)
    x_quarter("q", 0)
    x_quarter("q", 1)
    mask_load(0, 0)
    load_stripe(nc.gpsimd.dma_start, wv_sb, w_dram["v"], C.DO, 0, 512)
    x_quarter("v", 0)
    mask_load(0, 1)
    load_stripe(nc.scalar.dma_start, wk_sb, w_dram["k"], C.DO, 128, 512)
    load_stripe(nc.sync.dma_start, wq_sb, w_dram["q"], C.DO, 128, 512)
    x_quarter("k", 1)
    x_quarter("v", 1)
    x_quarter("k", 2)
    load_stripe(nc.gpsimd.dma_start, wo_sb, w_dram["o"], C.DM, 0, 1024)
    nc.sync.dma_start(R_sb[:], rrow[0:1, :].partition_broadcast(128))

    # ---------------- emitter closures ----------------
    def proj_block(which, hp, ns, half=None):
        """One psum sub-block of the k/q projection for pair hp, reading x
        from the rotating quarter buffer. half=None does the full 512-wide
        quarter (~1.7us of PE); half 0/1 does 256 columns (~0.85us claims,
        halving how long an interleaved unit stalls the scores rotation)."""
        w_sb, S, b_sb, dstT = {
            "k": (wk_sb, C.SK, bkl_sb, kT_sb),
            "q": (wq_sb, C.SQS, bql_sb, qT_sb),
        }[which]
        x_sb = xbuf[which][ns // 512]
        if which == "q":
            ns = ns % C.SQS  # qT keeps one sq-half resident at a time
        c0 = 0 if half is None else half * 256
        nw = 512 if half is None else 256
        ps = ps_s.tile([128, nw], F32, name=f"ps_{which}p", tag="s",
                       padded_shape=[128, PS_F])
        for kt in range(C.KT):
            nc.tensor.matmul(
                ps[:],
                w_sb[:, kt * C.DO + hp * 128: kt * C.DO + (hp + 1) * 128],
                x_sb[:, kt * 512 + c0: kt * 512 + c0 + nw],
                start=(kt == 0), stop=(kt == C.KT - 1),
            )
        nc.vector.tensor_scalar_add(
            dstT[:, hp * S + ns + c0: hp * S + ns + c0 + nw],
            ps[:], b_sb[:, hp:hp + 1])

    def vproj_j(jj):
        """v projection for one 128-key tile jj (one ~1.7us psum claim)."""
        xt = xbuf["v"][jj // 4]
        lj = jj % 4
        ps = ps_s.tile([128, C.DO], F32, name="ps_v", tag="s",
                       padded_shape=[128, PS_F])
        for kt in range(C.KT):
            nc.tensor.matmul(
                ps[:],
                xt[:, kt * 512 + lj * 128: kt * 512 + (lj + 1) * 128],
                wv_sb[:, kt * C.DO:(kt + 1) * C.DO],
                start=(kt == 0), stop=(kt == C.KT - 1),
            )
        nc.vector.tensor_copy(
            v_view[:, jj, 0:C.H, 0:64],
            ps.rearrange("p (h c) -> p h c", c=DK),
        )

    def oproj_block(m, ns, pool_tag=None):
        """Output projection sub-block: seq rows m*128..(m+1)*128, dm cols
        ns..ns+512, +R via DVE add on psum evacuation. The tail passes
        alternating psum pools so four claims pipeline."""
        nw = 512
        pl, tag = pool_tag or (ps_s, "s")
        ps = pl.tile([128, nw], F32, name="ps_o", tag=tag,
                     padded_shape=[128, PS_F])
        for kt in range(C.KO):
            nc.tensor.matmul(
                ps[:],
                xattnT_sb[:, kt * C.SQ + m * 128: kt * C.SQ + (m + 1) * 128],
                wo_sb[:, kt * C.DM + ns: kt * C.DM + ns + nw],
                start=(kt == 0), stop=(kt == C.KO - 1),
            )
        ot = attn.tile([128, nw], BF16, name="out_sb", tag="out_sb", bufs=2,
                       padded_shape=[128, nw])
        nc.vector.tensor_tensor(out=ot[:], in0=ps[:], in1=R_sb[:, ns:ns + nw],
                                op=ALU.add)
        nc.sync.dma_start(out[m * 128:(m + 1) * 128, ns:ns + nw], ot[:])

    def finish():
        pass

    # ---------------- pre-attention: minimal hp0/sq0 work ----------------
    proj_block("k", 0, 0)
    proj_block("q", 0, 0)
    proj_block("q", 0, 512)

    if C.max_stage <= 2:
        finish()
        return

    # per-(block, j) unit schedule: each unit is a ~1.7us PE claim placed
    # against its DMA arrival (earliest) and consumer deadline (latest)
    K_, Q_, V_, O_, XK, XQ, XV = "k", "q", "v", "o", "xk", "xq", "xv"
    sched = {
        0: {0: [(V_, 0), (V_, 1)],
            1: [(V_, 2), (K_, 1, 0)],
            2: [(V_, 3), (K_, 0, 512)],
            3: [(XV, 2), (V_, 4), (K_, 2, 0)],
            4: [(V_, 5), (K_, 3, 0)],
            5: [(XK, 3), (V_, 6), (Q_, 1, 0)],
            6: [(V_, 7), (Q_, 1, 512)],
            7: [(XV, 3), (V_, 8), (K_, 0, 1024)],
            8: [(V_, 9)],
            9: [(V_, 10), (K_, 0, 1536)],
            10: [(V_, 11)], 11: [(V_, 12)], 12: [(V_, 13)],
            13: [(V_, 14)], 14: [(V_, 15)]},
        1: {0: [(K_, 1, 512)], 1: [(K_, 1, 1024)], 2: [(K_, 1, 1536)],
            3: [(K_, 2, 512)], 4: [(Q_, 2, 0)], 5: [(Q_, 2, 512)]},
        2: {0: [(K_, 2, 1024)], 1: [(K_, 2, 1536)], 2: [(K_, 3, 512)],
            3: [(Q_, 3, 0)], 4: [(Q_, 3, 512)],
            5: [(XQ, 2)], 6: [(XQ, 3)], 7: [(K_, 3, 1024)]},
        3: {0: [(K_, 3, 1536)],
            5: [(Q_, 0, 1024)], 6: [(Q_, 0, 1536)]},
        4: {3: [(Q_, 1, 1024)], 4: [(Q_, 1, 1536)],
            5: [(O_, 0, 0)], 6: [(O_, 0, 512)],
            7: [(O_, 1, 0)], 8: [(O_, 1, 512)]},
        5: {3: [(Q_, 2, 1024)], 4: [(Q_, 2, 1536)],
            5: [(O_, 2, 0)], 6: [(O_, 2, 512)],
            7: [(O_, 3, 0)], 8: [(O_, 3, 512)]},
        6: {3: [(Q_, 3, 1024)], 4: [(Q_, 3, 1536)],
            5: [(O_, 4, 0)], 6: [(O_, 4, 512)],
            7: [(O_, 5, 0)], 8: [(O_, 5, 512)]},
        7: {5: [(O_, 6, 0)], 6: [(O_, 6, 512)],
            7: [(O_, 7, 0)], 8: [(O_, 7, 512)]},
    }
    if C.max_stage <= 3:
        for blk in sched.values():
            for js in blk.values():
                js[:] = [u for u in js if u[0] != O_]

    def run_unit(u):
        if u[0] == K_:
            proj_block("k", u[1], u[2], u[3] if len(u) > 3 else None)
        elif u[0] == Q_:
            proj_block("q", u[1], u[2], u[3] if len(u) > 3 else None)
        elif u[0] == V_:
            vproj_j(u[1])
        elif u[0] == O_:
            oproj_block(u[1], u[2])
        else:
            x_quarter({XK: "k", XQ: "q", XV: "v"}[u[0]], u[1],
                      dma=nc.sync.dma_start if u[0] != XV else None)

    PIPE = 2
    pending_fin = None
    for bi, (sq, hp) in enumerate((sq, hp) for (sq, _) in slices(C.SQ, C.SQS)
                                  for hp in range(C.HP)):
        sw = C.SQS
        pv = None
        pm_hist = []
        bsched = sched.get(bi, {})

        def emit_pv(jj, pms, hp=hp):
            for i in range(2):
                for (qs, qw) in slices(sw, 512):
                    nc.tensor.matmul(
                        pv[i][:, qs:qs + qw], v_view[:, jj, 2 * hp + i, :],
                        pms[i][:, qs:qs + qw],
                        start=(jj == 0), stop=(jj == C.NJ - 1),
                    )

        for j in range(C.NJ):
            if j == 1 and pending_fin is not None:
                pending_fin(0)
            if j == PIPE:
                # previous block's deferred finalize frees the pv slots the
                # first emit_pv below will claim (split across j=1/j=2 so
                # the DVE chain overlaps the mask stream)
                if pending_fin is not None:
                    pending_fin(1)
                    pending_fin = None
                pv = [
                    ps_pv.tile([65, sw], F32, name=f"ps_pv{i}", tag="pv",
                               padded_shape=[128, PS_F])
                    for i in range(2)
                ]
            if j % JG == 0:
                # mask DMA: block 0 tops up sq-half 0 (g0/g1 preloaded);
                # block 3 streams sq-half 1 groups as its own reads of each
                # group retire; block 4 fetches the last one
                if bi == 0 and j // JG + 2 < C.NJ // JG:
                    mask_load(0, j // JG + 2)
                elif bi == C.HP - 1 and j >= JG:
                    mask_load(1, j // JG - 1)
                elif bi == C.HP and j == 0:
                    mask_load(1, C.NJ // JG - 1)
            pms = []
            sss = [ps_s.tile([128, sw], F32, name=f"ps_sc{i}", tag="s",
                             padded_shape=[128, PS_F]) for i in range(2)]
            # interleave the two heads' MMs so the row-tiled (0,0)/(64,0)
            # pairs sit adjacent in the PE queue and run concurrently
            for (qs, qw) in slices(sw, 512):
                for i in range(2):
                    nc.tensor.matmul(
                        sss[i][:, qs:qs + qw],
                        kT_sb[i * 64:(i + 1) * 64,
                              hp * C.SK + j * 128: hp * C.SK + (j + 1) * 128],
                        qT_sb[i * 64:(i + 1) * 64,
                              hp * C.SQS + qs: hp * C.SQS + qs + qw],
                        start=True, stop=True,
                    )
            for i in range(2):
                pe = attn.tile([128, sw], BF16, name="p_exp", tag="pexp",
                               bufs=3, padded_shape=[128, C.SQS])
                nc.scalar.activation(pe[:], sss[i][:], AF.Exp)
                pm = attn.tile([128, sw], BF16, name="p_msk", tag="pmask",
                               bufs=6, padded_shape=[128, C.SQS])
                nc.vector.tensor_tensor(
                    out=pm[:], in0=pe[:],
                    in1=mask_sb[:, j * C.SQS:(j + 1) * C.SQS],
                    op=ALU.mult,
                )
                pms.append(pm)
            pm_hist.append((j, pms))
            if len(pm_hist) > PIPE:
                jj, pp = pm_hist.pop(0)
                emit_pv(jj, pp)
            for u in bsched.get(j, []):
                run_unit(u)
        for jj, pp in pm_hist:
            emit_pv(jj, pp)

        def finalize(i, pv=pv, hp=hp, sq=sq):
            if True:
                # Z row -> sbuf, 1/Z in place (DVE; the approx-recip custom
                # op must not read PSUM directly), GpSimd partition-
                # broadcast to 64 rows, then the per-element divide on DVE
                zr1 = attn.tile([1, sw], F32, name="zr1", tag="zr1",
                                bufs=1, padded_shape=[1, C.SQS])
                nc.vector.tensor_copy(zr1[:], pv[i][64:65, :])
                nc.vector.reciprocal_approx_fast(out=zr1[:], in_=zr1[:])
                tmp = attn.tile([64, sw], BF16, name="xat_t", tag="xat_t",
                                bufs=1, padded_shape=[64, C.SQS])
                for (qs, qw) in slices(sw, 512):
                    zrow = attn.tile([64, qw], F32, name="zrow", tag="zrow",
                                     bufs=1, padded_shape=[64, 512])
                    nc.gpsimd.partition_broadcast(zrow[:], zr1[:, qs:qs + qw],
                                                  channels=64)
                    nc.vector.tensor_tensor(out=tmp[:, qs:qs + qw],
                                            in0=pv[i][0:64, qs:qs + qw],
                                            in1=zrow[:], op=ALU.mult)
                # partition hop: rows 0..63 -> xattnT pair rows 64i..64i+64
                nc.sync.dma_start(
                    xattnT_sb[64 * i:64 * (i + 1),
                              hp * C.SQ + sq: hp * C.SQ + sq + sw],
                    tmp[:],
                )

        pending_fin = finalize
    pending_fin(0)
    pending_fin(1)

    if C.max_stage <= 3:
        finish()
        return

    # ---------------- tail: remaining output projection ----------------
    # tail runs entirely on the "pv" psum slots: the "s" rotation is left
    # free so a following rep's projections/scores claims start immediately
    for m in range(C.SQ // 256, C.SQ // 128):
        for ns in (0, 512):
            oproj_block(m, ns, pool_tag=(ps_pv, "pv"))

    finish()


def build(cfg, reps=1):
    nc = bacc.Bacc("TRN2", target_bir_lowering=False, debug=False)
    C = cfg
    io = {
        "xqT": nc.dram_tensor("xqT", [C.DM, C.SQ], BF16, kind="ExternalInput").ap(),
        "xkT": nc.dram_tensor("xkT", [C.DM, C.SK], BF16, kind="ExternalInput").ap(),
        "xvT": nc.dram_tensor("xvT", [C.DM, C.SK], BF16, kind="ExternalInput").ap(),
        "maskT": nc.dram_tensor("maskT", [(C.SQ // C.SQS) * C.SK, C.SQS], BF16,
                            kind="ExternalInput").ap(),
        "wqt": nc.dram_tensor("wqt", [C.DM, C.DO], BF16, kind="ExternalInput").ap(),
        "wkt": nc.dram_tensor("wkt", [C.DM, C.DO], BF16, kind="ExternalInput").ap(),
        "wvt": nc.dram_tensor("wvt", [C.DM, C.DO], BF16, kind="ExternalInput").ap(),
        "wot": nc.dram_tensor("wot", [C.DO, C.DM], BF16, kind="ExternalInput").ap(),
        "bql": nc.dram_tensor("bql", [128, C.HP], F32, kind="ExternalInput").ap(),
        "bkl": nc.dram_tensor("bkl", [128, C.HP], F32, kind="ExternalInput").ap(),
        "rrow": nc.dram_tensor("rrow", [1, C.DM], BF16, kind="ExternalInput").ap(),
        "out": nc.dram_tensor("out", [C.SQ, C.DM], BF16, kind="ExternalOutput").ap(),
    }
    with tile.TileContext(nc) as tc:
        state = {}
        for _ in range(reps):
            emit_kernel(tc, cfg, io, state)
        for pl in reversed(list(state["pools"].values())):
            pl.release()
    nc.compile()
    return nc


def host_prep(query, key, value, mask, Wq, bq, Wk, bk, Wv, bv, Wo, bo, cfg):
    """Host-side layout prep (transpose/cast, per-core slicing)."""
    C = cfg
    bf = ml_dtypes.bfloat16
    wqt_full = (Wq.T * 0.125).astype(bf)     # 1/sqrt(dk) folded
    wkt_full = Wk.T.astype(bf)
    wvt_full = Wv.T.astype(bf)
    wot_full = Wo.T.astype(bf)
    bqs = (bq * 0.125).astype(np.float32)
    in_maps = []
    for c in range(N_CORES):
        b, h = divmod(c, 2)
        d0, d1 = h * C.DO, (h + 1) * C.DO
        rrow = (bv[d0:d1].astype(np.float64) @ Wo.T[d0:d1].astype(np.float64)
                + (bo.astype(np.float64) if h == 0 else 0.0))
        m = {
            "xqT": np.ascontiguousarray(query[b].T.astype(bf)),
            "xkT": np.ascontiguousarray(key[b].T.astype(bf)),
            "xvT": np.ascontiguousarray(value[b].T.astype(bf)),
            "maskT": np.ascontiguousarray(
                mask[b].T.astype(bf).reshape(C.NJ, 128, C.SQ // C.SQS, C.SQS)
                .transpose(2, 0, 1, 3).reshape(-1, C.SQS)),
            "wqt": np.ascontiguousarray(wqt_full[:, d0:d1]),
            "wkt": np.ascontiguousarray(wkt_full[:, d0:d1]),
            "wvt": np.ascontiguousarray(wvt_full[:, d0:d1]),
            "wot": np.ascontiguousarray(wot_full[d0:d1, :]),
            "bql": np.ascontiguousarray(bqs[d0:d1].reshape(C.HP, 128).T),
            "bkl": np.ascontiguousarray(
                bk[d0:d1].astype(np.float32).reshape(C.HP, 128).T),
            "rrow": np.ascontiguousarray(
                rrow.reshape(1, C.DM)).astype(ml_dtypes.bfloat16),
        }
        in_maps.append(m)
    return in_maps


_CACHED = {}


def get_built():
    if "nc" not in _CACHED:
        _CACHED["nc"] = build(Cfg())
    return _CACHED["nc"]


def kernel(query, key, value, mask, Wq, bq, Wk, bk, Wv, bv, Wo, bo):
    cfg = Cfg()
    nc = get_built()
    in_maps = host_prep(query, key, value, mask, Wq, bq, Wk, bk, Wv, bv, Wo, bo, cfg)
    res = run_bass_kernel_spmd(nc, in_maps, core_ids=list(range(N_CORES)))
    B, S, DM = query.shape
    out = np.empty((B, S, DM), np.float32)
    for b in range(B):
        out[b] = (res.results[2 * b]["out"].astype(np.float32)
                  + res.results[2 * b + 1]["out"].astype(np.float32))
    return out
